# revision 6
# baseline (speedup 1.0000x reference)
"""Trainium2 Bass kernel for nn_CBL_1632087573343 (boundary context loss).

Data-parallel over batch: 8 images -> 8 NeuronCores, one image per core.

Per-core pipeline (one image), Gram-matrix formulation:
  - er is host-cast to bf16 and packed into 8 range-slabs
    [2 chunks, 4 row-ranges, 128, RSLAB] so each 32-row band (plus 2
    rows of read-ahead) is an independent SBUF tile, letting the PE
    start before the full image has loaded.
  - For every row y and 32-pixel group g the PE computes a narrow Gram
    block G[m, 36r + w] = dot_c(er[:, y, 32g+m], er[:, y+r, 32g-2+w])
    (contraction over the 128-channel chunks, accumulated in PSUM,
    tile_position=(0, 32g) stacks the 4 groups in PE array columns).
    Every cosine numerator AND the squared-norm field are diagonals of
    these blocks -- no elementwise product pass and no one-hot
    reduction is needed at all.
  - Diagonals cannot be extracted on-chip (engine reads are
    partition-uniform; SBUF DMA partition-step drift wraps mod 16 B),
    so the blocks bounce through a DRAM scratch: PSUM->SBUF copy
    (ACT/DVE alternating), contiguous write at row pitch SM=112, and a
    stride-113 readback that turns the shear into a legal strided DMA.
    One readback per (group, 32-row range) lands all 13 dot-fields in
    [x, 80*y + c] layout (c = 36*dy + dx + 2).
  - Labels (lab) and fold weights (W = valid + valid_s) are
    host-computed in transposed [x, y] layout; the device only does the
    tiny pointwise phase: cos = dot*rn*rn_s, (cos-lab)^2 * W, reduce.
Device returns S_i = sum_s sum_p W_s (cos_s - lab_s)^2; host computes
loss = sum_i [S_i / max(cnt_i,1) / 24 * include_i] / max(sum include, 1).
"""

import sys

sys.path.insert(0, "/opt/trn_rl_repo")

import numpy as np

import concourse.bass as bass
import concourse.tile as tile
from concourse import bacc, mybir

DT = mybir.dt
F32 = DT.float32
BF16 = DT.bfloat16
ALU = mybir.AluOpType
AX = mybir.AxisListType

B, C, H, W = 8, 256, 128, 128
NR = 4                           # row ranges (32 rows each + 2 readahead)
RROWS = 34                       # rows resident per range tile
RSLAB = 2 + RROWS * W + 130      # 4484: front pad 2, back pad
SM = 112                         # DRAM scratch row pitch (per pixel m)
SG = 32 * SM                     # 3584: per (y, group) block
SY = 4 * SG                      # 14336: per y
RB = 77                          # readback window: c = 36*dy + dx + 2
FYP = 80                         # fld per-y pitch (>= RB)
FP = FYP * H                     # 10240: fld cols per partition
LH = 128                         # labw per-plane pitch

# canonical half of the 24-shift set (mirror folded into W on host)
SHIFTS = [(0, 1), (0, 2), (1, -2), (1, -1), (1, 0), (1, 1), (1, 2),
          (2, -2), (2, -1), (2, 0), (2, 1), (2, 2)]


def _ap(t, offset, dims):
    return bass.AP(t.tensor, offset, [list(d) for d in dims])


def build_kernel(nc):
    er_d = nc.dram_tensor("ers", [2, NR, 128, RSLAB], BF16,
                          kind="ExternalInput")
    lw_d = nc.dram_tensor("labw", [128, 24 * LH], BF16,
                          kind="ExternalInput")
    out_d = nc.dram_tensor("out", [1, 2], F32, kind="ExternalOutput")

    with tile.TileContext(nc) as tc:
        _build(tc, er_d, lw_d, out_d)
    nc.compile()
    return nc


def _build(tc, er_d, lw_d, out_d):
    nc = tc.nc
    from contextlib import ExitStack

    with ExitStack() as ctx:
        const_p = ctx.enter_context(tc.tile_pool(name="const", bufs=1))
        er_p = ctx.enter_context(tc.tile_pool(name="erp", bufs=1))
        g4_p = ctx.enter_context(tc.tile_pool(name="g4p", bufs=3))
        fld_p = ctx.enter_context(tc.tile_pool(name="fldp", bufs=1))
        scr_p = ctx.enter_context(tc.tile_pool(name="scrp", bufs=1))
        psum_p = ctx.enter_context(
            tc.tile_pool(name="psump", bufs=4, space="PSUM"))
        dram_p = ctx.enter_context(
            tc.tile_pool(name="dramp", bufs=1, space="DRAM"))

        ones_f = const_p.tile([128, 16], F32, name="ones_f", tag="ones_f")
        nc.vector.memset(ones_f[:], 1.0)
        R = const_p.tile([128, 64], F32, name="R", tag="R")
        nc.vector.memset(R[:], 0.0)

        labw = const_p.tile([128, 24 * LH], BF16, name="labw", tag="labw")
        nc.scalar.dma_start(out=labw[:], in_=lw_d.ap())

        # ---- er range-slab loads (sync ring, range-major) --------------
        er = [[None] * NR for _ in range(2)]
        for r in range(NR):
            for c in range(2):
                e = er_p.tile([128, RSLAB], BF16, name=f"er{c}_{r}",
                              tag=f"er{c}_{r}")
                nc.sync.dma_start(
                    out=e[:],
                    in_=_ap(er_d.ap(), (c * NR + r) * 128 * RSLAB,
                            [[RSLAB, 128], [1, RSLAB]]))
                er[c][r] = e

        fld = fld_p.tile([128, FP], BF16, name="fld", tag="fld")
        scratch = [dram_p.tile([1, 32 * SY], BF16, name=f"scr{r}",
                               tag=f"scr{r}") for r in range(NR)]

        # persistent rn tiles: rn[x, y] and its 4 partition-shifted
        # copies rd[dx][x, y] = rn[x+dx, y]; filled 32 y-cols per range
        rn = const_p.tile([128, 132], BF16, name="rn", tag="rn")
        nc.vector.memset(rn[:], 0.0)
        rshift = {0: rn}
        for dx in (-2, -1, 1, 2):
            t = const_p.tile([128, 132], BF16, name=f"rn_d{dx}",
                             tag=f"rn_d{dx}")
            nc.vector.memset(t[:], 0.0)
            rshift[dx] = t

        pw_p = ctx.enter_context(tc.tile_pool(name="pwp", bufs=2))
        Y0 = (0, 30, 62, 94)
        Y1 = (30, 62, 94, 128)

        def _pointwise_range(ri):
            y0, y1 = Y0[ri], Y1[ri]
            n = y1 - y0
            w0, w1 = 32 * ri, 32 * ri + 32
            # rn window: 1/max(sqrt(norm2), eps) over new 32 y-cols
            rn1 = pw_p.tile([128, 40], F32, name="rn1", tag="rn1")
            nc.scalar.sqrt(rn1[:, 0:32],
                           _ap(fld, 2 + FYP * w0, [[FP, 128], [FYP, 32]]))
            nc.vector.tensor_scalar(rn1[:, 0:32], rn1[:, 0:32], 1e-8,
                                    None, op0=ALU.max)
            rnf = pw_p.tile([128, 40], F32, name="rnf", tag="rnf")
            nc.vector.reciprocal(rnf[:, 0:32], rn1[:, 0:32])
            nc.vector.tensor_copy(rn[:, w0:w1], rnf[:, 0:32])
            for dx in (-2, -1, 1, 2):
                t = rshift[dx]
                if dx > 0:
                    nc.scalar.dma_start(out=t[0:128 - dx, w0:w1],
                                        in_=rn[dx:128, w0:w1])
                else:
                    nc.scalar.dma_start(out=t[-dx:128, w0:w1],
                                        in_=rn[0:128 + dx, w0:w1])
            for i, (dy, dx) in enumerate(SHIFTS):
                c_idx = 36 * dy + dx + 2
                fldp = _ap(fld, c_idx + FYP * y0,
                           [[FP, 128], [FYP, n]])
                t1 = pw_p.tile([128, 40], BF16, name="t1", tag="t1")
                nc.vector.tensor_tensor(t1[:, 0:n], fldp, rn[:, y0:y1],
                                        op=ALU.mult)
                rs = rshift[dx]
                cosb = pw_p.tile([128, 40], BF16, name="cosb",
                                 tag="cosb")
                nc.vector.tensor_tensor(cosb[:, 0:n], t1[:, 0:n],
                                        rs[:, y0 + dy:y1 + dy],
                                        op=ALU.mult)
                d = pw_p.tile([128, 40], BF16, name="d", tag="d")
                nc.vector.tensor_tensor(
                    d[:, 0:n], cosb[:, 0:n],
                    labw[:, 2 * i * LH + y0:2 * i * LH + y1],
                    op=ALU.subtract)
                e2 = pw_p.tile([128, 40], BF16, name="e2", tag="e2")
                nc.scalar.square(e2[:, 0:n], d[:, 0:n])
                fw = pw_p.tile([128, 40], BF16, name="fw", tag="fw")
                nc.vector.tensor_tensor(
                    fw[:, 0:n], e2[:, 0:n],
                    labw[:, (2 * i + 1) * LH + y0:(2 * i + 1) * LH + y1],
                    op=ALU.mult)
                nc.vector.tensor_reduce(R[:, 12 * ri + i:12 * ri + i + 1],
                                        fw[:, 0:n], axis=AX.X, op=ALU.add)

        # ---- main loop: 32 blocks of 4 rows ----------------------------
        for yb in range(32):
            ri = yb // 8
            ps = psum_p.tile([128, 512], F32, name="ps", tag="ps")
            for q in range(4):
                y = 4 * yb + q
                ry = y - 32 * ri
                nrows = min(3, 128 - y)
                for g in range(4):
                    for c in range(2):
                        base = 2 + ry * W + 32 * g
                        st = er[c][ri][:, base:base + 32]
                        mov = _ap(er[c][ri], base - 2,
                                  [[RSLAB, 128], [W, nrows], [1, 36]])
                        nc.tensor.matmul(
                            ps[32 * g:32 * g + 32,
                               108 * q:108 * q + 36 * nrows],
                            st, mov, start=(c == 0), stop=(c == 1),
                            skip_group_check=True,
                            tile_position=(0, 32 * g))
            g4 = g4_p.tile([128, 432], BF16, name="g4", tag="g4")
            if yb % 2 == 0:
                nc.scalar.copy(g4[:], ps[0:128, 0:432])
            else:
                nc.vector.tensor_copy(g4[:], ps[0:128, 0:432])
            # scratch write: addr = y_local*SY + p*SM + col
            nc.scalar.dma_start(
                out=_ap(scratch[ri], (yb % 8) * 4 * SY,
                        [[SM, 128], [SY, 4], [1, 108]]),
                in_=_ap(g4, 0, [[432, 128], [108, 4], [1, 108]]))
            if yb % 8 == 7:
                # shear readback: (m, y_local, c) from
                # y_local*SY + g*SG + 113*m + c -> fld[32g+m, 80*y + c]
                # on the SWDGE (gpsimd) queue to decongest the rings
                for g in range(4):
                    nc.gpsimd.dma_start(
                        out=_ap(fld, 32 * g * FP + FYP * 32 * ri,
                                [[FP, 32], [FYP, 32], [1, RB]]),
                        in_=_ap(scratch[ri], g * SG,
                                [[113, 32], [SY, 32], [1, RB]]))
                _pointwise_range(ri)

        # ---- final reduction: S = sum over R columns & partitions ------
        ps2 = psum_p.tile([128, 512], F32, name="ps2", tag="ps")
        nc.tensor.matmul(ps2[0:1, 0:48], ones_f[:, 0:1], R[:, 0:48],
                         start=True, stop=True)
        scal = scr_p.tile([1, 64], F32, name="scal", tag="scal")
        nc.scalar.copy(scal[0:1, 0:48], ps2[0:1, 0:48])
        nc.vector.tensor_reduce(scal[0:1, 48:49], scal[0:1, 0:48],
                                axis=AX.X, op=ALU.add)

        outt = scr_p.tile([1, 32], F32, name="outt", tag="outt")
        nc.vector.tensor_copy(outt[0:1, 0:1], scal[0:1, 48:49])
        nc.vector.memset(outt[0:1, 1:2], 0.0)
        nc.sync.dma_start(out=out_d.ap(), in_=outt[0:1, 0:2])


_NC_CACHE = {}


def get_nc():
    if "nc" not in _NC_CACHE:
        nc = bacc.Bacc("TRN2", target_bir_lowering=False, debug=False)
        build_kernel(nc)
        _NC_CACHE["nc"] = nc
    return _NC_CACHE["nc"]


def _prep_slabs(er):
    """er f32 [B, C, H, W] -> bf16 range slabs [B, 2, NR, 128, RSLAB]."""
    import ml_dtypes

    erb = np.ascontiguousarray(er.reshape(B, 2, 128, H * W)).astype(
        ml_dtypes.bfloat16)
    ers = np.zeros((B, 2, NR, 128, RSLAB), dtype=ml_dtypes.bfloat16)
    for r in range(NR):
        lo = 32 * r * W
        hi = min((32 * r + RROWS) * W, H * W)
        ers[:, :, r, :, 2:2 + hi - lo] = erb[:, :, :, lo:hi]
    return ers


def _prep_labels(seg, gtb):
    """Host label prep in transposed [x, y] layout: labw [B, 128, 24*LH]
    bf16 plus (cnt, include) per image."""
    import ml_dtypes

    seg0 = np.where(seg == 255, 0, seg)
    gtb0 = np.where(gtb == 255, 0, gtb)
    gt_b = (gtb0 * seg0).astype(np.int64)            # [B, H, W]
    interior = np.zeros((H, W), bool)
    interior[2:H - 2, 2:W - 2] = True
    valid = (gt_b > 0) & interior                    # [B, H, W]
    include = (gt_b > 0).any(axis=(1, 2)).astype(np.float64)
    cnt = valid.sum(axis=(1, 2)).astype(np.float64)

    labw = np.zeros((B, 128, 24 * LH), dtype=ml_dtypes.bfloat16)
    vf = valid.astype(np.float32)
    for s_i, (dy, dx) in enumerate(SHIFTS):
        seg_s = np.roll(seg, (-dy, -dx), axis=(1, 2))
        lab = ((seg == seg_s) & (seg < 2)).astype(np.float32)
        v_s = np.zeros_like(vf)
        v_s[:, :H - dy, :] = vf[:, dy:, :]
        w = np.zeros_like(vf)
        if dx >= 0:
            w[:, :, :W - dx] = v_s[:, :, dx:]
        else:
            w[:, :, -dx:] = v_s[:, :, :W + dx]
        w += vf
        # transpose [y, x] -> [x, y]
        labw[:, :, 2 * s_i * LH:2 * s_i * LH + 128] = \
            lab.transpose(0, 2, 1)
        labw[:, :, (2 * s_i + 1) * LH:(2 * s_i + 1) * LH + 128] = \
            w.transpose(0, 2, 1)
    return labw, cnt, include


def kernel(er_input, seg_label, gt_boundary_seg):
    er = np.ascontiguousarray(np.asarray(er_input, dtype=np.float32))
    seg = np.ascontiguousarray(np.asarray(seg_label, dtype=np.int32))
    gtb = np.ascontiguousarray(np.asarray(gt_boundary_seg, dtype=np.int32))
    assert er.shape == (B, C, H, W), er.shape

    ers = _prep_slabs(er)
    labw, cnt, include = _prep_labels(seg, gtb)
    nc = get_nc()
    from concourse.bass_utils import run_bass_kernel_spmd

    in_maps = [
        {"ers": ers[i], "labw": labw[i]} for i in range(B)
    ]
    res = run_bass_kernel_spmd(nc, in_maps, list(range(B)))
    S = np.array([res.results[i]["out"][0, 0] for i in range(B)],
                 dtype=np.float64)
    loss_i = S / np.maximum(cnt, 1.0) / 24.0 * include
    loss = loss_i.sum() / max(include.sum(), 1.0)
    return np.float32(loss)


# revision 7
# speedup vs baseline: 1.1000x; 1.1000x over previous
"""Trainium2 Bass kernel for nn_CBL_1632087573343 (boundary context loss).

Data-parallel over batch: 8 images -> 8 NeuronCores, one image per core.

Per-core pipeline (one image), Gram-matrix formulation:
  - er is host-cast to bf16 and packed into 8 range-slabs
    [2 chunks, 4 row-ranges, 128, RSLAB] so each 32-row band (plus 2
    rows of read-ahead) is an independent SBUF tile, letting the PE
    start before the full image has loaded.
  - For every row y and 32-pixel group g the PE computes a narrow Gram
    block G[m, 36r + w] = dot_c(er[:, y, 32g+m], er[:, y+r, 32g-2+w])
    (contraction over the 128-channel chunks, accumulated in PSUM,
    tile_position=(0, 32g) stacks the 4 groups in PE array columns).
    Every cosine numerator AND the squared-norm field are diagonals of
    these blocks -- no elementwise product pass and no one-hot
    reduction is needed at all.
  - Diagonals cannot be extracted on-chip (engine reads are
    partition-uniform; SBUF DMA partition-step drift wraps mod 16 B),
    so the blocks bounce through a DRAM scratch: PSUM->SBUF copy (ACT),
    contiguous write at row pitch SM=112, and a stride-113 readback
    that turns the shear into a legal strided DMA.  One readback per
    (group, 32-row range) lands all 13 dot-fields in per-range tiles
    fld[x, 80*y_local + c] (c = 36*dy + dx + 2).  Range tiles overlap
    by one 4-row block (boundary G-write duplicated) so each range's
    pointwise phase is independent -- no cross-range WAR serialization.
  - Pointwise per range, overlapped with the next range's matmuls:
    rn = 1/max(sqrt(norm2), eps); per shift cos = dot*rn*rn_s and
    w = cos*(W*cos + Bh) with host planes W = valid + valid_s and
    Bh = -2*W*lab; the constant sum(W*lab^2) is added on the host.
    The first multiply rides GPSIMD, the rest DVE; one packed
    tensor_reduce per range.
Device returns S_i = sum_s sum_p [W cos^2 - 2 W lab cos]; host adds
C_i = sum(W lab^2) and computes
loss = sum_i [(S_i + C_i) / max(cnt_i,1) / 24 * include_i] / scale_num.
"""

import sys

sys.path.insert(0, "/opt/trn_rl_repo")

import numpy as np

import concourse.bass as bass
import concourse.tile as tile
from concourse import bacc, mybir

DT = mybir.dt
F32 = DT.float32
BF16 = DT.bfloat16
ALU = mybir.AluOpType
AX = mybir.AxisListType

B, C, H, W = 8, 256, 128, 128
NR = 4                           # row ranges (32 rows each + 2 readahead)
RROWS = 34                       # rows resident per range tile
RSLAB = 2 + RROWS * W + 130      # 4484: front pad 2, back pad
SM = 112                         # DRAM scratch row pitch (per pixel m)
SG = 32 * SM                     # 3584: per (y, group) block
SY = 4 * SG                      # 14336: per y
RB = 77                          # readback window: c = 36*dy + dx + 2
FYP = 80                         # fld per-y pitch (>= RB)
LH = 128                         # labw per-plane pitch

# canonical half of the 24-shift set (mirror folded into W on host)
SHIFTS = [(0, 1), (0, 2), (1, -2), (1, -1), (1, 0), (1, 1), (1, 2),
          (2, -2), (2, -1), (2, 0), (2, 1), (2, 2)]

# pointwise windows [PY0, PY1) per range; fld tile ri covers y from
# FW0[ri] (ranges overlap by the duplicated boundary 4-row block)
PY0 = (0, 30, 62, 94)
PY1 = (30, 62, 94, 128)
FW0 = (0, 28, 60, 92)
FNY = (32, 36, 36, 36)           # y rows per fld/scratch range tile


def _ap(t, offset, dims):
    return bass.AP(t.tensor, offset, [list(d) for d in dims])


def build_kernel(nc):
    er_d = nc.dram_tensor("ers", [2, NR, 128, RSLAB], BF16,
                          kind="ExternalInput")
    lw_d = nc.dram_tensor("labw", [128, 24 * LH], BF16,
                          kind="ExternalInput")
    out_d = nc.dram_tensor("out", [1, 2], F32, kind="ExternalOutput")

    with tile.TileContext(nc) as tc:
        _build(tc, er_d, lw_d, out_d)
    nc.compile()
    return nc


def _build(tc, er_d, lw_d, out_d):
    nc = tc.nc
    from contextlib import ExitStack

    with ExitStack() as ctx:
        const_p = ctx.enter_context(tc.tile_pool(name="const", bufs=1))
        er_p = ctx.enter_context(tc.tile_pool(name="erp", bufs=1))
        g4_p = ctx.enter_context(tc.tile_pool(name="g4p", bufs=3))
        fld_p = ctx.enter_context(tc.tile_pool(name="fldp", bufs=1))
        scr_p = ctx.enter_context(tc.tile_pool(name="scrp", bufs=1))
        pw_p = ctx.enter_context(tc.tile_pool(name="pwp", bufs=2))
        psum_p = ctx.enter_context(
            tc.tile_pool(name="psump", bufs=4, space="PSUM"))
        dram_p = ctx.enter_context(
            tc.tile_pool(name="dramp", bufs=1, space="DRAM"))

        ones_f = const_p.tile([128, 16], F32, name="ones_f", tag="ones_f")
        nc.vector.memset(ones_f[:], 1.0)
        R = const_p.tile([128, 8], F32, name="R", tag="R")
        nc.vector.memset(R[:], 0.0)

        labw = const_p.tile([128, 24 * LH], BF16, name="labw", tag="labw")
        nc.scalar.dma_start(out=labw[:], in_=lw_d.ap())

        # ---- er range-slab loads (sync ring, range-major) --------------
        er = [[None] * NR for _ in range(2)]
        for r in range(NR):
            for c in range(2):
                e = er_p.tile([128, RSLAB], BF16, name=f"er{c}_{r}",
                              tag=f"er{c}_{r}")
                nc.sync.dma_start(
                    out=e[:],
                    in_=_ap(er_d.ap(), (c * NR + r) * 128 * RSLAB,
                            [[RSLAB, 128], [1, RSLAB]]))
                er[c][r] = e

        fld = [fld_p.tile([128, FYP * FNY[r]], BF16, name=f"fld{r}",
                          tag=f"fld{r}") for r in range(NR)]
        scratch = [dram_p.tile([1, FNY[r] * SY], BF16, name=f"scr{r}",
                               tag=f"scr{r}") for r in range(NR)]

        # persistent rn tiles: rn[x, y] and its 4 partition-shifted
        # copies rd[dx][x, y] = rn[x+dx, y]; filled 32 y-cols per range
        rn = const_p.tile([128, 132], BF16, name="rn", tag="rn")
        nc.vector.memset(rn[:], 0.0)
        rshift = {0: rn}
        for dx in (-2, -1, 1, 2):
            t = const_p.tile([128, 132], BF16, name=f"rn_d{dx}",
                             tag=f"rn_d{dx}")
            nc.vector.memset(t[:], 0.0)
            rshift[dx] = t

        def _pointwise_range(ri):
            y0, y1 = PY0[ri], PY1[ri]
            n = y1 - y0
            w0, w1 = 32 * ri, 32 * ri + 32
            f = fld[ri]
            fny = FNY[ri]
            # rn window: 1/max(sqrt(norm2), eps) over new 32 y-cols
            rn1 = pw_p.tile([128, 40], F32, name="rn1", tag="rn1")
            nc.scalar.sqrt(
                rn1[:, 0:32],
                _ap(f, 2 + FYP * (w0 - FW0[ri]),
                    [[FYP * fny, 128], [FYP, 32]]))
            nc.vector.tensor_scalar(rn1[:, 0:32], rn1[:, 0:32], 1e-8,
                                    None, op0=ALU.max)
            rnf = pw_p.tile([128, 40], F32, name="rnf", tag="rnf")
            nc.vector.reciprocal(rnf[:, 0:32], rn1[:, 0:32])
            nc.vector.tensor_copy(rn[:, w0:w1], rnf[:, 0:32])
            for dx in (-2, -1, 1, 2):
                t = rshift[dx]
                if dx > 0:
                    nc.scalar.dma_start(out=t[0:128 - dx, w0:w1],
                                        in_=rn[dx:128, w0:w1])
                else:
                    nc.scalar.dma_start(out=t[-dx:128, w0:w1],
                                        in_=rn[0:128 + dx, w0:w1])
            wbuf = pw_p.tile([128, 12 * 34], BF16, name="wbuf",
                             tag="wbuf")
            for i, (dy, dx) in enumerate(SHIFTS):
                c_idx = 36 * dy + dx + 2
                fldp = _ap(f, c_idx + FYP * (y0 - FW0[ri]),
                           [[FYP * fny, 128], [FYP, n]])
                t1 = pw_p.tile([128, 40], BF16, name="t1", tag="t1")
                nc.gpsimd.tensor_tensor(t1[:, 0:n], fldp, rn[:, y0:y1],
                                        op=ALU.mult)
                rs = rshift[dx]
                cosb = pw_p.tile([128, 40], BF16, name="cosb",
                                 tag="cosb")
                nc.vector.tensor_tensor(cosb[:, 0:n], t1[:, 0:n],
                                        rs[:, y0 + dy:y1 + dy],
                                        op=ALU.mult)
                v1 = pw_p.tile([128, 40], BF16, name="v1", tag="v1")
                nc.vector.tensor_tensor(
                    v1[:, 0:n], cosb[:, 0:n],
                    labw[:, (2 * i + 1) * LH + y0:(2 * i + 1) * LH + y1],
                    op=ALU.mult)
                v2 = pw_p.tile([128, 40], BF16, name="v2", tag="v2")
                nc.vector.tensor_tensor(
                    v2[:, 0:n], v1[:, 0:n],
                    labw[:, 2 * i * LH + y0:2 * i * LH + y1],
                    op=ALU.add)
                nc.vector.tensor_tensor(wbuf[:, i * n:(i + 1) * n],
                                        cosb[:, 0:n], v2[:, 0:n],
                                        op=ALU.mult)
            nc.vector.tensor_reduce(R[:, ri:ri + 1], wbuf[:, 0:12 * n],
                                    axis=AX.X, op=ALU.add)

        # ---- main loop: 32 blocks of 4 rows ----------------------------
        for yb in range(32):
            ri = yb // 8
            ps = psum_p.tile([128, 512], F32, name="ps", tag="ps")
            for q in range(4):
                y = 4 * yb + q
                ry = y - 32 * ri
                nrows = min(3, 128 - y)
                for g in range(4):
                    for c in range(2):
                        base = 2 + ry * W + 32 * g
                        st = er[c][ri][:, base:base + 32]
                        mov = _ap(er[c][ri], base - 2,
                                  [[RSLAB, 128], [W, nrows], [1, 36]])
                        nc.tensor.matmul(
                            ps[32 * g:32 * g + 32,
                               108 * q:108 * q + 36 * nrows],
                            st, mov, start=(c == 0), stop=(c == 1),
                            skip_group_check=True,
                            tile_position=(0, 32 * g))
            g4 = g4_p.tile([128, 432], BF16, name="g4", tag="g4")
            nc.scalar.copy(g4[:], ps[0:128, 0:432])
            # scratch write: slot = 4*(yb%8) (+4 head room for ri>=1)
            slot = 4 * (yb % 8) + (4 if ri >= 1 else 0)
            nc.scalar.dma_start(
                out=_ap(scratch[ri], slot * SY,
                        [[SM, 128], [SY, 4], [1, 108]]),
                in_=_ap(g4, 0, [[432, 128], [108, 4], [1, 108]]))
            if yb % 8 == 7:
                if yb < 31:
                    # duplicate boundary block into next range's head
                    nc.scalar.dma_start(
                        out=_ap(scratch[ri + 1], 0,
                                [[SM, 128], [SY, 4], [1, 108]]),
                        in_=_ap(g4, 0, [[432, 128], [108, 4], [1, 108]]))
                # shear readback: (m, y_local, c) from
                # y_local*SY + g*SG + 113*m + c
                #   -> fld[ri][32g+m, 80*y_local + c]
                fny = FNY[ri]
                for g in range(4):
                    eng = nc.gpsimd if g < 2 else nc.sync
                    eng.dma_start(
                        out=_ap(fld[ri], 32 * g * FYP * fny,
                                [[FYP * fny, 32], [FYP, fny], [1, RB]]),
                        in_=_ap(scratch[ri], g * SG,
                                [[113, 32], [SY, fny], [1, RB]]))
                _pointwise_range(ri)

        # ---- final reduction: S = sum over R columns & partitions ------
        ps2 = psum_p.tile([128, 512], F32, name="ps2", tag="ps")
        nc.tensor.matmul(ps2[0:1, 0:4], ones_f[:, 0:1], R[:, 0:4],
                         start=True, stop=True)
        scal = scr_p.tile([1, 8], F32, name="scal", tag="scal")
        nc.scalar.copy(scal[0:1, 0:4], ps2[0:1, 0:4])
        nc.vector.tensor_reduce(scal[0:1, 4:5], scal[0:1, 0:4],
                                axis=AX.X, op=ALU.add)

        outt = scr_p.tile([1, 8], F32, name="outt", tag="outt")
        nc.vector.tensor_copy(outt[0:1, 0:1], scal[0:1, 4:5])
        nc.vector.memset(outt[0:1, 1:2], 0.0)
        nc.sync.dma_start(out=out_d.ap(), in_=outt[0:1, 0:2])


_NC_CACHE = {}


def get_nc():
    if "nc" not in _NC_CACHE:
        nc = bacc.Bacc("TRN2", target_bir_lowering=False, debug=False)
        build_kernel(nc)
        _NC_CACHE["nc"] = nc
    return _NC_CACHE["nc"]


def _prep_slabs(er):
    """er f32 [B, C, H, W] -> bf16 range slabs [B, 2, NR, 128, RSLAB]."""
    import ml_dtypes

    erb = np.ascontiguousarray(er.reshape(B, 2, 128, H * W)).astype(
        ml_dtypes.bfloat16)
    ers = np.zeros((B, 2, NR, 128, RSLAB), dtype=ml_dtypes.bfloat16)
    for r in range(NR):
        lo = 32 * r * W
        hi = min((32 * r + RROWS) * W, H * W)
        ers[:, :, r, :, 2:2 + hi - lo] = erb[:, :, :, lo:hi]
    return ers


def _prep_labels(seg, gtb):
    """Host label prep in transposed [x, y] layout: labw [B, 128, 24*LH]
    bf16 holding (Bh = -2*W*lab, W) per shift, plus per-image
    (cnt, include, Ch = sum W*lab^2)."""
    import ml_dtypes

    seg0 = np.where(seg == 255, 0, seg)
    gtb0 = np.where(gtb == 255, 0, gtb)
    gt_b = (gtb0 * seg0).astype(np.int64)            # [B, H, W]
    interior = np.zeros((H, W), bool)
    interior[2:H - 2, 2:W - 2] = True
    valid = (gt_b > 0) & interior                    # [B, H, W]
    include = (gt_b > 0).any(axis=(1, 2)).astype(np.float64)
    cnt = valid.sum(axis=(1, 2)).astype(np.float64)

    labw = np.zeros((B, 128, 24 * LH), dtype=ml_dtypes.bfloat16)
    ch = np.zeros(B, dtype=np.float64)
    vf = valid.astype(np.float32)
    for s_i, (dy, dx) in enumerate(SHIFTS):
        seg_s = np.roll(seg, (-dy, -dx), axis=(1, 2))
        lab = ((seg == seg_s) & (seg < 2)).astype(np.float32)
        v_s = np.zeros_like(vf)
        v_s[:, :H - dy, :] = vf[:, dy:, :]
        w = np.zeros_like(vf)
        if dx >= 0:
            w[:, :, :W - dx] = v_s[:, :, dx:]
        else:
            w[:, :, -dx:] = v_s[:, :, :W + dx]
        w += vf
        ch += (w * lab * lab).sum(axis=(1, 2)).astype(np.float64)
        # transpose [y, x] -> [x, y]; plane A = Bh, plane B = W
        labw[:, :, 2 * s_i * LH:2 * s_i * LH + 128] = \
            (-2.0 * w * lab).transpose(0, 2, 1)
        labw[:, :, (2 * s_i + 1) * LH:(2 * s_i + 1) * LH + 128] = \
            w.transpose(0, 2, 1)
    return labw, cnt, include, ch


def kernel(er_input, seg_label, gt_boundary_seg):
    er = np.ascontiguousarray(np.asarray(er_input, dtype=np.float32))
    seg = np.ascontiguousarray(np.asarray(seg_label, dtype=np.int32))
    gtb = np.ascontiguousarray(np.asarray(gt_boundary_seg, dtype=np.int32))
    assert er.shape == (B, C, H, W), er.shape

    ers = _prep_slabs(er)
    labw, cnt, include, ch = _prep_labels(seg, gtb)
    nc = get_nc()
    from concourse.bass_utils import run_bass_kernel_spmd

    in_maps = [
        {"ers": ers[i], "labw": labw[i]} for i in range(B)
    ]
    res = run_bass_kernel_spmd(nc, in_maps, list(range(B)))
    S = np.array([res.results[i]["out"][0, 0] for i in range(B)],
                 dtype=np.float64)
    loss_i = (S + ch) / np.maximum(cnt, 1.0) / 24.0 * include
    loss = loss_i.sum() / max(include.sum(), 1.0)
    return np.float32(loss)
